# revision 1
# baseline (speedup 1.0000x reference)
"""Multi-head attention (B=2, T=2048, C=2048, H=16, causal, rotary) on 8
Trainium2 NeuronCores.

Sharding: tensor-parallel over heads x data-parallel over batch.
Core c handles batch b = c // 4 and heads [4*(c%4), 4*(c%4)+4).
Each core computes a partial output y_c = attn_out(4 heads) @ wo_rows;
the host sums the 4 partials per batch (row-parallel wo).

v2 design (vs the DRAM-spill baseline):
  - everything in bf16 on the PE (same 1 cycle/col rate as f32r, half
    the DMA + SBUF footprint); psum accumulation stays fp32.
  - Q^T/K^T (RoPE'd, de-interleaved) and V live entirely in SBUF --
    no DRAM spill/reload.
  - projection t-chunks and attention q-chunks are interleaved:
    proj(t) then attn(qc=t) (which needs keys only up to chunk t),
    then wo(qc=t).  The PE never waits on a phase boundary.
  - attention inner loop is software-pipelined depth-2 (QK of block
    kb+2 issues before AV/rowsum of block kb) so the in-order PE queue
    never head-of-line blocks on the ACT exp.
  - softmax normalization: rowsum via ones-stationary matmul (psum
    accumulated over kb), reciprocal on DVE, partition_broadcast on
    GPSIMD (no PE broadcast matmul, no extra psum bank).
  - psum banks: pp(proj+wo)=2, pss(scores)=3, pso(AV)=2, psr(rowsum)=1.
"""

import math
import os
import sys
from contextlib import ExitStack

import numpy as np

for _p in ("/opt/trn_rl_repo", "/root/.axon_site/_ro/trn_rl_repo"):
    if os.path.isdir(_p) and _p not in sys.path:
        sys.path.append(_p)

import bass_rust
import ml_dtypes
import concourse.bass as bass
import concourse.mybir as mybir
import concourse.tile as tile
from concourse import library_config
from concourse.bass_utils import run_bass_kernel_spmd
from concourse.vector_clock import ScopedClock, VectorClock

B, T, C, H = 2, 2048, 2048, 16
D = 128
HPC = H // 4          # 4 heads per core
DH = HPC * D          # 512 head-dims per core
NCH = C // 128        # 16 contraction chunks
TCH = 512             # t-chunk == q-chunk
NT = T // TCH         # 4
N_CORES = 8
SCALE = 1.0 / math.sqrt(D)

f32 = mybir.dt.float32
f32r = mybir.dt.float32r
bf16 = mybir.dt.bfloat16
AF = mybir.ActivationFunctionType
BF16NP = ml_dtypes.bfloat16


# --------------------------------------------------------------------------
# toolchain workarounds (from the known-good baseline)
# --------------------------------------------------------------------------
def _patched_drain_and_barrier(self, tick_clock, wait_clock):
    """walrus codegen accepts only one sem wait on an InstDrain; emit one
    drain per outstanding proc instead of one drain with N waits."""
    ticks = list(tick_clock.global_clock)
    for i, t in enumerate(ticks):
        if t <= 0:
            continue
        sub = VectorClock([t if j == i else 0 for j in range(len(ticks))])
        d = self.nc.sync.drain()
        wait_clock.add_sem_waits(d.ins, ScopedClock({None: sub}))
    self.nc.all_engine_barrier()
    assert self.sems is not None
    popped = self.nc._tile_sem_poison_stack.pop()
    assert popped is self._sem_poison
    self.nc.clear_and_free_semaphores(list(self.sems.allocated().values()))
    self.nc.all_engine_barrier()


tile.TileContext._drain_and_barrier = _patched_drain_and_barrier

_SPLIT_OPS = {
    "Matmult", "Drain", "DMACopy", "DMATransposeAnt", "Activation", "TensorTensor", "TensorReduce",
    "TensorCopy", "Reciprocal", "TensorScalarPtr", "TensorScalar",
    "CopyPredicated", "Memset", "NoOp", "Pool", "Max", "MaxIndex",
    "StreamShuffle", "StreamTranspose", "TensorTensorScan",
    "ScalarTensorTensor", "TensorTensorReduce", "Iota", "BNStats",
    "BNStatsAggregate", "Select", "PartitionBroadcast",
}
_ws_counter = [0]


def _split_waits(nc, limit=1):
    """walrus encodes a limited number of sem waits on engine instructions
    (fused bf16 LDW+MM and Drain take only one). Move excess waits onto
    same-engine NoOps inserted immediately before; engine program order
    preserves semantics."""
    for f in nc.m.functions:
        for b in f.blocks:
            insts = b.instructions
            i = 0
            while i < len(insts):
                inst = insts[i]
                si = inst.sync_info
                if (
                    inst.opcode not in _SPLIT_OPS
                    or si is None
                    or not si.on_wait
                    or len(si.on_wait) <= limit
                ):
                    i += 1
                    continue
                waits = list(si.on_wait)
                extra, keep = waits[:-limit], waits[-limit:]
                for w in extra:
                    _ws_counter[0] += 1
                    nop = bass_rust.InstNoOp(
                        name=f"I-waitsplit-{_ws_counter[0]}", engine=inst.engine
                    )
                    nop.sync_info = mybir.SyncInfo(on_wait=[w], on_update=[])
                    insts.insert(i, nop)
                    i += 1
                inst.sync_info = mybir.SyncInfo(
                    on_wait=keep,
                    on_update=list(si.on_update) if si.on_update else [],
                )
                i += 1


# --------------------------------------------------------------------------
# kernel build
# --------------------------------------------------------------------------
def _build_nc():
    nc = bass.Bass("TRN2", debug=False, target_bir_lowering=False)

    xT = nc.dram_tensor("xT", [C, T], bf16, kind="ExternalInput").ap()
    wq = nc.dram_tensor("wq", [C, DH], bf16, kind="ExternalInput").ap()
    wk = nc.dram_tensor("wk", [C, DH], bf16, kind="ExternalInput").ap()
    wv = nc.dram_tensor("wv", [C, DH], bf16, kind="ExternalInput").ap()
    wo = nc.dram_tensor("wo", [DH, C], bf16, kind="ExternalInput").ap()
    cosT = nc.dram_tensor("cosT", [64, T], bf16, kind="ExternalInput").ap()
    sinT = nc.dram_tensor("sinT", [64, T], bf16, kind="ExternalInput").ap()
    mbd = nc.dram_tensor("mb", [128, 128], bf16, kind="ExternalInput").ap()
    onesk_d = nc.dram_tensor("onesk", [128, 1], bf16, kind="ExternalInput").ap()
    ones1_d = nc.dram_tensor("ones1", [1, 128], f32r, kind="ExternalInput").ap()
    y = nc.dram_tensor("y", [T, C], bf16, kind="ExternalOutput").ap()

    with tile.TileContext(nc) as tc, ExitStack() as es:
        # ---- pools (whole-kernel lifetime) ----
        wpool = es.enter_context(tc.tile_pool(name="w", bufs=1))
        wopool = es.enter_context(tc.tile_pool(name="wo", bufs=1))
        xpool = es.enter_context(tc.tile_pool(name="x", bufs=2))
        cpool = es.enter_context(tc.tile_pool(name="cs", bufs=1))
        persist = es.enter_context(tc.tile_pool(name="qkv", bufs=1))
        rt = es.enter_context(tc.tile_pool(name="rt", bufs=2))
        ep = es.enter_context(tc.tile_pool(name="e", bufs=6))
        otp = es.enter_context(tc.tile_pool(name="ot", bufs=2))
        rp = es.enter_context(tc.tile_pool(name="r", bufs=2))
        ysbp = es.enter_context(tc.tile_pool(name="ysb", bufs=2))

        pp = es.enter_context(tc.tile_pool(name="pp", bufs=2, space="PSUM"))
        pss = es.enter_context(tc.tile_pool(name="pss", bufs=3, space="PSUM"))
        pso = es.enter_context(tc.tile_pool(name="pso", bufs=2, space="PSUM"))
        psr = es.enter_context(tc.tile_pool(name="psr", bufs=1, space="PSUM"))

        # ---- persistent SBUF tensors ----
        qT = {h: persist.tile([128, T], bf16, tag=f"qT{h}", name=f"qT{h}") for h in range(HPC)}
        kT = {h: persist.tile([128, T], bf16, tag=f"kT{h}", name=f"kT{h}") for h in range(HPC)}
        v_all = persist.tile([128, NT * 4 * DH], bf16, tag="v_all", name="v_all")
        # v_all[:, kb*512 + h*128 : kb*512 + (h+1)*128] = V rows of key-block
        # kb for head h: [k=128, d=128] -- one [128,512] ACT copy per block

        w_tiles = {}
        cos_t = cpool.tile([64, T], bf16, tag="cos")
        sin_t = cpool.tile([64, T], bf16, tag="sin")
        mb_t = cpool.tile([128, 128], bf16, tag="mb")
        onesk = cpool.tile([128, 1], bf16, tag="onesk")
        ones1 = cpool.tile([1, 128], f32r, tag="ones1")
        wo_tiles = []

        ot_tiles = {}
        pending = [None]

        def _emit_norm(h, pso_t, psr_t):
            # 1/rowsum broadcast: copy rowsums to SBUF (f32r), replicate
            # across partitions with a ones-stationary matmul into a pp psum
            # bank (pp is otherwise idle during attention), reciprocal on
            # DVE, then multiply out of psum (one PSUM operand per DVE op).
            rsc = rp.tile([1, TCH], f32r, tag="rsc")
            nc.scalar.copy(rsc[:], psr_t[:])
            psb = pp.tile([128, TCH], f32, tag="pp")
            nc.tensor.matmul(psb[:], ones1[:], rsc[:], start=True, stop=True)
            binv = rp.tile([128, TCH], f32, tag="binv")
            nc.vector.reciprocal(binv[:], psb[:])
            ot = otp.tile([128, TCH], bf16, tag=f"ot{h}")
            nc.vector.tensor_mul(ot[:], pso_t[:], binv[:])
            ot_tiles[h] = ot

        def _emit_wo_group(qc, qs):
            # one q-subblock of the deferred output projection; interleaved
            # after each attention head so the PE fills ACT-limited stretches
            ysb_t = ysbp.tile([128, C], bf16, tag="ysb")
            for cc in range(C // 512):
                psy = pp.tile([128, 512], f32, tag="pp")
                for hh in range(HPC):
                    nc.tensor.matmul(
                        psy[:],
                        prev_ot[hh][:, qs * 128:(qs + 1) * 128],
                        wo_tiles[hh][:, cc * 512:(cc + 1) * 512],
                        start=(hh == 0),
                        stop=(hh == HPC - 1),
                    )
                if qc == NT - 1:
                    nc.scalar.copy(ysb_t[:, cc * 512:(cc + 1) * 512], psy[:])
                else:
                    nc.vector.tensor_copy(
                        ysb_t[:, cc * 512:(cc + 1) * 512], psy[:]
                    )
            row0 = qc * TCH + qs * 128
            nc.sync.dma_start(y[row0:row0 + 128, :], ysb_t[:])

        def _rope(ps, dst_h, tsl):
            c_sl = cos_t[:, tsl]
            s_sl = sin_t[:, tsl]
            t1 = rt.tile([64, TCH], f32, tag="r1")
            nc.vector.tensor_mul(t1[:], ps[0:64, :], c_sl)
            t2 = rt.tile([64, TCH], f32, tag="r2")
            nc.vector.tensor_mul(t2[:], ps[64:128, :], s_sl)
            nc.vector.tensor_sub(dst_h[0:64, tsl], t1[:], t2[:])
            t3 = rt.tile([64, TCH], f32, tag="r3")
            nc.vector.tensor_mul(t3[:], ps[0:64, :], s_sl)
            t4 = rt.tile([64, TCH], f32, tag="r4")
            nc.vector.tensor_mul(t4[:], ps[64:128, :], c_sl)
            nc.vector.tensor_add(dst_h[64:128, tsl], t3[:], t4[:])

        next_xt = None
        prev_ot = None

        def _emit_qk_proj(t, xt, t0=False):
            tsl_ = bass.ts(t, TCH)
            if t0:
                # run 7 projection groups concurrently across the (still
                # idle) attention psum pools so the PE keeps pace with the
                # w/x DMA stream: one matmul per group per arriving chunk.
                _gp = [("wq", 0, pp, "pp"), ("wq", 1, pss, "s"),
                       ("wq", 2, pss, "s"), ("wq", 3, pss, "s"),
                       ("wk", 0, pso, "o"), ("wk", 1, pso, "o"),
                       ("wk", 2, pp, "pp")]
                groups = [
                    (wn, h_, pool.tile([128, TCH], f32, tag=tg, name=f"g{t}_{wn}{h_}"))
                    for wn, h_, pool, tg in _gp
                ]
                for ci in range(NCH):
                    for wname, h, ps in groups:
                        nc.tensor.matmul(
                            ps[:],
                            w_tiles[(wname, ci)][:, h * 128:(h + 1) * 128],
                            xt[ci][:],
                            start=(ci == 0),
                            stop=(ci == NCH - 1),
                        )
                for wname, h, ps in groups:
                    _rope(ps, (qT if wname == "wq" else kT)[h], tsl_)
                # K-h3 is emitted interleaved with the wv-paced V(0)-tsi0
                # group (see _emit_v_proj) so the PE has resident-data work
                # while wv streams in.
                kh3_pend.append(True)
            else:
                for wname, dst in (("wq", qT), ("wk", kT)):
                    for h in range(HPC):
                        ps = pp.tile([128, TCH], f32, tag="pp")
                        for ci in range(NCH):
                            nc.tensor.matmul(
                                ps[:],
                                w_tiles[(wname, ci)][:, h * 128:(h + 1) * 128],
                                xt[ci][:],
                                start=(ci == 0),
                                stop=(ci == NCH - 1),
                            )
                        _rope(ps, dst[h], tsl_)

        kh3_pend = []

        def _emit_v_proj(t, xt):
            psk3 = None
            if kh3_pend:
                kh3_pend.clear()
                psk3 = pp.tile([128, TCH], f32, tag="pp", name="psk3")
            for tsi in range(TCH // 128):
                ps = pp.tile([128, DH], f32, tag="pp")
                for ci in range(NCH):
                    nc.tensor.matmul(
                        ps[:],
                        xt[ci][:, tsi * 128:(tsi + 1) * 128],
                        w_tiles[("wv", ci)][:],
                        start=(ci == 0),
                        stop=(ci == NCH - 1),
                    )
                    if psk3 is not None and tsi == 0:
                        nc.tensor.matmul(
                            psk3[:],
                            w_tiles[("wk", ci)][:, 3 * 128:4 * 128],
                            xt[ci][:],
                            start=(ci == 0), stop=(ci == NCH - 1),
                        )
                if psk3 is not None and tsi == 0:
                    _rope(psk3, kT[3], bass.ts(t, TCH))
                kb = t * 4 + tsi
                nc.vector.tensor_copy(v_all[:, kb * DH:(kb + 1) * DH], ps[:])

        def _prefetch_x(t):
            nsl = bass.ts(t, TCH)
            tiles = []
            for ci in range(NCH):
                x_ = xpool.tile([128, TCH], bf16, tag=f"x{ci}")
                nc.scalar.dma_start(x_[:], xT[ci * 128:(ci + 1) * 128, nsl])
                tiles.append(x_)
            return tiles

        # ---------------- t0: stream everything in ----------------
        nc.sync.dma_start(mb_t[:], mbd)
        nc.sync.dma_start(onesk[:], onesk_d)
        nc.sync.dma_start(ones1[:], ones1_d)
        xt_cur = []
        for ci in range(NCH):
            wt = wpool.tile([128, DH], bf16, tag=f"wq{ci}")
            nc.sync.dma_start(wt[:], wq[ci * 128:(ci + 1) * 128, :])
            w_tiles[("wq", ci)] = wt
            x_ = xpool.tile([128, TCH], bf16, tag=f"x{ci}")
            nc.scalar.dma_start(x_[:], xT[ci * 128:(ci + 1) * 128, 0:TCH])
            xt_cur.append(x_)
            wt = wpool.tile([128, DH], bf16, tag=f"wk{ci}")
            nc.gpsimd.dma_start(wt[:], wk[ci * 128:(ci + 1) * 128, :])
            w_tiles[("wk", ci)] = wt
        nc.scalar.dma_start(cos_t[:], cosT)
        nc.scalar.dma_start(sin_t[:], sinT)
        # wv split across the two w queues so it lands as the V matmuls start
        for ci in range(NCH):
            wt = wpool.tile([128, DH], bf16, tag=f"wv{ci}")
            eng = nc.sync if ci % 2 == 0 else nc.gpsimd
            eng.dma_start(wt[:], wv[ci * 128:(ci + 1) * 128, :])
            w_tiles[("wv", ci)] = wt
        _emit_qk_proj(0, xt_cur, t0=True)
        xt_next = _prefetch_x(1)
        _emit_v_proj(0, xt_cur)
        for j in range(HPC):
            wt_ = wopool.tile([128, C], bf16, tag=f"wo{j}")
            nc.sync.dma_start(wt_[:], wo[j * 128:(j + 1) * 128, :])
            wo_tiles.append(wt_)

        # ------- steady pipeline: attn(t) + wo(t-1), QK(t+1), V(t+1) -------
        for t in range(NT):
            if t >= 1:
                xt_cur = xt_next
                xt_next = _prefetch_x(t + 1) if t + 1 < NT else None
            qc = t
            kmax = 4 * qc + 3
            for h in range(HPC):
                q_sl = qT[h][:, qc * TCH:(qc + 1) * TCH]
                pso_t = pso.tile([128, TCH], f32, tag="o")
                psr_t = psr.tile([1, TCH], f32, tag="rs")
                av_q = []  # pending blocks for depth-2 pipelined AV/rowsum

                def _emit_av(h=h, pso_t=pso_t, psr_t=psr_t, kmax=kmax):
                    kb, qlo, e = av_q.pop(0)
                    nc.tensor.matmul(
                        pso_t[:, qlo:],
                        v_all[:, kb * DH + h * 128:kb * DH + (h + 1) * 128],
                        e[:, qlo:],
                        start=(kb == 0), stop=(kb == kmax),
                    )
                    nc.tensor.matmul(
                        psr_t[:, qlo:], onesk[:], e[:, qlo:],
                        start=(kb == 0), stop=(kb == kmax),
                    )

                for kb in range(kmax + 1):
                    i_rel = kb - 4 * qc
                    qlo = 128 * i_rel if i_rel > 0 else 0
                    pss_t = pss.tile([128, TCH], f32, tag="s")
                    nc.tensor.matmul(
                        pss_t[:, qlo:],
                        kT[h][:, kb * 128:(kb + 1) * 128],
                        q_sl[:, qlo:],
                        start=True,
                        stop=True,
                    )
                    e = ep.tile([128, TCH], bf16, tag="e")
                    nc.scalar.activation(
                        e[:, qlo:], pss_t[:, qlo:], AF.Exp, scale=SCALE
                    )
                    if i_rel >= 0:  # triangle mask on the diagonal square
                        nc.gpsimd.tensor_mul(
                            e[:, qlo:qlo + 128],
                            e[:, qlo:qlo + 128],
                            mb_t[:],
                        )
                    av_q.append((kb, qlo, e))
                    if kb == 1 and pending[0] is not None:
                        _emit_norm(*pending[0])
                        pending[0] = None
                    if len(av_q) > 3:
                        _emit_av()
                while av_q:
                    _emit_av()
                pending[0] = (h, pso_t, psr_t)
                if prev_ot is not None:
                    _emit_wo_group(qc - 1, h)
            _emit_norm(*pending[0])
            pending[0] = None
            prev_ot = dict(ot_tiles)
            ot_tiles = {}
            if t + 1 < NT:
                _emit_qk_proj(t + 1, xt_next)
                _emit_v_proj(t + 1, xt_next)

        for qs in range(TCH // 128):
            _emit_wo_group(NT - 1, qs)

    _split_waits(nc)
    return nc


_CACHED_NC = None


def _get_nc():
    global _CACHED_NC
    if _CACHED_NC is None:
        _CACHED_NC = _build_nc()
    return _CACHED_NC


# --------------------------------------------------------------------------
# host-side input prep / gather
# --------------------------------------------------------------------------
def _deinterleave_perm():
    """per-head column permutation: [2j for j<64] then [2j+1]"""
    p = np.empty(D, dtype=np.int64)
    p[:64] = np.arange(0, D, 2)
    p[64:] = np.arange(1, D, 2)
    return p


def _make_core_inputs(x, freqs_cos, freqs_sin, wq, wk, wv, wo):
    x = np.asarray(x, dtype=np.float32)
    freqs_cos = np.asarray(freqs_cos, dtype=np.float32)
    freqs_sin = np.asarray(freqs_sin, dtype=np.float32)
    wq = np.asarray(wq, dtype=np.float32)
    wk = np.asarray(wk, dtype=np.float32)
    wv = np.asarray(wv, dtype=np.float32)
    wo = np.asarray(wo, dtype=np.float32)

    perm = _deinterleave_perm()
    cosT = np.ascontiguousarray(freqs_cos.T).astype(BF16NP)  # [64, T]
    sinT = np.ascontiguousarray(freqs_sin.T).astype(BF16NP)

    # causal triangle for the diagonal 128x128 square: mb[k, q] = 1 iff k <= q
    k_idx = np.arange(128)[:, None]
    q_idx = np.arange(128)[None, :]
    mb = (k_idx <= q_idx).astype(BF16NP)

    onesk = np.ones((128, 1), dtype=BF16NP)
    ones1 = np.ones((1, 128), dtype=np.float32)

    xTb = [np.ascontiguousarray(x[b].T).astype(BF16NP) for b in range(B)]

    in_maps = []
    for core in range(N_CORES):
        b, hg = core // 4, core % 4
        cols = slice(hg * DH, (hg + 1) * DH)
        wq_s = wq[:, cols].reshape(C, HPC, D)[:, :, perm].reshape(C, DH)
        wk_s = wk[:, cols].reshape(C, HPC, D)[:, :, perm].reshape(C, DH)
        in_maps.append({
            "xT": xTb[b],
            "wq": np.ascontiguousarray(wq_s).astype(BF16NP),
            "wk": np.ascontiguousarray(wk_s).astype(BF16NP),
            "wv": np.ascontiguousarray(wv[:, cols]).astype(BF16NP),
            "wo": np.ascontiguousarray(wo[cols, :]).astype(BF16NP),
            "cosT": cosT,
            "sinT": sinT,
            "mb": mb,
            "onesk": onesk,
            "ones1": ones1,
        })
    return in_maps


def kernel(x, freqs_cos, freqs_sin, wq, wk, wv, wo, _trace=False, _trace_kwargs=None):
    nc = _get_nc()
    in_maps = _make_core_inputs(x, freqs_cos, freqs_sin, wq, wk, wv, wo)
    res = run_bass_kernel_spmd(
        nc, in_maps, core_ids=list(range(N_CORES)), trace=_trace,
        **(_trace_kwargs or {}),
    )
    out = np.zeros((B, T, C), dtype=np.float32)
    for core in range(N_CORES):
        out[core // 4] += np.asarray(res.results[core]["y"], dtype=np.float32)
    if _trace:
        kernel.last_results = res
    return out



# revision 4
# speedup vs baseline: 1.0034x; 1.0034x over previous
"""Multi-head attention (B=2, T=2048, C=2048, H=16, causal, rotary) on 8
Trainium2 NeuronCores.

Sharding: tensor-parallel over heads x data-parallel over batch.
Core c handles batch b = c // 4 and heads [4*(c%4), 4*(c%4)+4).
Each core computes a partial output y_c = attn_out(4 heads) @ wo_rows;
the host sums the 4 partials per batch (row-parallel wo).

v3 design (vs v2, which hit 414us with PE 86% busy):
  - proj(t+1) and V(t+1) are emitted PER HEAD inside attention(t)'s head
    loop, so attention's exp-dependent stretches always have independent
    PE work queued behind them (v2 emitted proj at chunk boundaries and
    stalled ~11us entering attention t=0).
  - rope is 2 DVE muls (against duplicated [128,T] cos/sin tables) + 2
    GpSimd add/subs instead of 6 DVE ops on [64,...] tiles: DVE busy was
    249us (within 100us of PE) and is the next serialization risk.
  - psum->sbuf copies (V tiles, wo output tiles) alternate DVE/ACT.
  - startup DMA order puts wq/wk/x first on their queues and moves wv to
    the previously unused vector queue; first matmul ~5us (was 11.6us).
  - x prefetch for t+1 is split across the scalar and sync queues.
  - psum banks: pp(proj+wo+norm bcast)=2, pss(scores)=3, pso(AV)=2,
    psr(rowsum)=1.
"""

import math
import os
import sys
from contextlib import ExitStack

import numpy as np

for _p in ("/opt/trn_rl_repo", "/root/.axon_site/_ro/trn_rl_repo"):
    if os.path.isdir(_p) and _p not in sys.path:
        sys.path.append(_p)

import bass_rust
import ml_dtypes
import concourse.bass as bass
import concourse.mybir as mybir
import concourse.tile as tile
from concourse import library_config
from concourse.bass_utils import run_bass_kernel_spmd
from concourse.vector_clock import ScopedClock, VectorClock

B, T, C, H = 2, 2048, 2048, 16
D = 128
HPC = H // 4          # 4 heads per core
DH = HPC * D          # 512 head-dims per core
NCH = C // 128        # 16 contraction chunks
TCH = 512             # t-chunk == q-chunk
NT = T // TCH         # 4
N_CORES = 8
SCALE = 1.0 / math.sqrt(D)

f32 = mybir.dt.float32
f32r = mybir.dt.float32r
bf16 = mybir.dt.bfloat16
AF = mybir.ActivationFunctionType
BF16NP = ml_dtypes.bfloat16


# --------------------------------------------------------------------------
# toolchain workarounds (from the known-good baseline)
# --------------------------------------------------------------------------
def _patched_drain_and_barrier(self, tick_clock, wait_clock):
    """walrus codegen accepts only one sem wait on an InstDrain; emit one
    drain per outstanding proc instead of one drain with N waits."""
    ticks = list(tick_clock.global_clock)
    for i, t in enumerate(ticks):
        if t <= 0:
            continue
        sub = VectorClock([t if j == i else 0 for j in range(len(ticks))])
        d = self.nc.sync.drain()
        wait_clock.add_sem_waits(d.ins, ScopedClock({None: sub}))
    self.nc.all_engine_barrier()
    assert self.sems is not None
    popped = self.nc._tile_sem_poison_stack.pop()
    assert popped is self._sem_poison
    self.nc.clear_and_free_semaphores(list(self.sems.allocated().values()))
    self.nc.all_engine_barrier()


tile.TileContext._drain_and_barrier = _patched_drain_and_barrier

_SPLIT_OPS = {
    "Matmult", "Drain", "DMACopy", "DMATransposeAnt", "Activation", "TensorTensor", "TensorReduce",
    "TensorCopy", "Reciprocal", "TensorScalarPtr", "TensorScalar",
    "CopyPredicated", "Memset", "NoOp", "Pool", "Max", "MaxIndex",
    "StreamShuffle", "StreamTranspose", "TensorTensorScan",
    "ScalarTensorTensor", "TensorTensorReduce", "Iota", "BNStats",
    "BNStatsAggregate", "Select", "PartitionBroadcast",
}
_ws_counter = [0]


def _split_waits(nc, limit=1):
    """walrus encodes a limited number of sem waits on engine instructions
    (fused bf16 LDW+MM and Drain take only one). Move excess waits onto
    same-engine NoOps inserted immediately before; engine program order
    preserves semantics."""
    for f in nc.m.functions:
        for b in f.blocks:
            insts = b.instructions
            i = 0
            while i < len(insts):
                inst = insts[i]
                si = inst.sync_info
                if (
                    inst.opcode not in _SPLIT_OPS
                    or si is None
                    or not si.on_wait
                    or len(si.on_wait) <= limit
                ):
                    i += 1
                    continue
                waits = list(si.on_wait)
                extra, keep = waits[:-limit], waits[-limit:]
                for w in extra:
                    _ws_counter[0] += 1
                    nop = bass_rust.InstNoOp(
                        name=f"I-waitsplit-{_ws_counter[0]}", engine=inst.engine
                    )
                    nop.sync_info = mybir.SyncInfo(on_wait=[w], on_update=[])
                    insts.insert(i, nop)
                    i += 1
                inst.sync_info = mybir.SyncInfo(
                    on_wait=keep,
                    on_update=list(si.on_update) if si.on_update else [],
                )
                i += 1


# --------------------------------------------------------------------------
# kernel build
# --------------------------------------------------------------------------
def _build_nc():
    nc = bass.Bass("TRN2", debug=False, target_bir_lowering=False)

    xT = nc.dram_tensor("xT", [C, T], bf16, kind="ExternalInput").ap()
    wq = nc.dram_tensor("wq", [C, DH], bf16, kind="ExternalInput").ap()
    wk = nc.dram_tensor("wk", [C, DH], bf16, kind="ExternalInput").ap()
    wv = nc.dram_tensor("wv", [C, DH], bf16, kind="ExternalInput").ap()
    wo = nc.dram_tensor("wo", [DH, C], bf16, kind="ExternalInput").ap()
    cos2 = nc.dram_tensor("cos2", [128, T], bf16, kind="ExternalInput").ap()
    sin2 = nc.dram_tensor("sin2", [128, T], bf16, kind="ExternalInput").ap()
    mbd = nc.dram_tensor("mb", [128, 128], bf16, kind="ExternalInput").ap()
    onesk_d = nc.dram_tensor("onesk", [128, 1], bf16, kind="ExternalInput").ap()
    ones1_d = nc.dram_tensor("ones1", [1, 128], f32r, kind="ExternalInput").ap()
    y = nc.dram_tensor("y", [T, C], bf16, kind="ExternalOutput").ap()

    with tile.TileContext(nc) as tc, ExitStack() as es:
        # ---- pools (whole-kernel lifetime) ----
        wpool = es.enter_context(tc.tile_pool(name="w", bufs=1))
        wopool = es.enter_context(tc.tile_pool(name="wo", bufs=1))
        xpool = es.enter_context(tc.tile_pool(name="x", bufs=2))
        cpool = es.enter_context(tc.tile_pool(name="cs", bufs=1))
        persist = es.enter_context(tc.tile_pool(name="qkv", bufs=1))
        rt = es.enter_context(tc.tile_pool(name="rt", bufs=2))
        ep = es.enter_context(tc.tile_pool(name="e", bufs=6))
        otp = es.enter_context(tc.tile_pool(name="ot", bufs=2))
        rp = es.enter_context(tc.tile_pool(name="r", bufs=2))
        ysbp = es.enter_context(tc.tile_pool(name="ysb", bufs=2))

        pp = es.enter_context(tc.tile_pool(name="pp", bufs=2, space="PSUM"))
        pss = es.enter_context(tc.tile_pool(name="pss", bufs=3, space="PSUM"))
        pso = es.enter_context(tc.tile_pool(name="pso", bufs=2, space="PSUM"))
        psr = es.enter_context(tc.tile_pool(name="psr", bufs=1, space="PSUM"))

        # ---- persistent SBUF tensors ----
        qT = {h: persist.tile([128, T], bf16, tag=f"qT{h}", name=f"qT{h}") for h in range(HPC)}
        kT = {h: persist.tile([128, T], bf16, tag=f"kT{h}", name=f"kT{h}") for h in range(HPC)}
        v_all = persist.tile([128, NT * 4 * DH], bf16, tag="v_all", name="v_all")
        # v_all[:, kb*512 + h*128 : kb*512 + (h+1)*128] = V rows of key-block
        # kb for head h: [k=128, d=128] -- one [128,512] copy per block

        w_tiles = {}
        cs2_t = cpool.tile([128, T], bf16, tag="cos")
        sn2_t = cpool.tile([128, T], bf16, tag="sin")
        mb_t = cpool.tile([128, 128], bf16, tag="mb")
        onesk = cpool.tile([128, 1], bf16, tag="onesk")
        ones1 = cpool.tile([1, 128], f32r, tag="ones1")
        wo_tiles = []

        ot_tiles = {}
        pending = [None]

        def _emit_norm(h, pso_t, psr_t):
            # 1/rowsum broadcast: copy rowsums to SBUF (f32r), replicate
            # across partitions with a ones-stationary matmul into a pp psum
            # bank, reciprocal on DVE, then multiply out of psum (one PSUM
            # operand per DVE op).
            rsc = rp.tile([1, TCH], f32r, tag="rsc")
            nc.scalar.copy(rsc[:], psr_t[:])
            psb = pp.tile([128, TCH], f32, tag="pp")
            nc.tensor.matmul(psb[:], ones1[:], rsc[:], start=True, stop=True)
            binv = rp.tile([128, TCH], f32, tag="binv")
            nc.vector.reciprocal(binv[:], psb[:])
            ot = otp.tile([128, TCH], bf16, tag=f"ot{h}")
            nc.vector.tensor_mul(ot[:], pso_t[:], binv[:])
            ot_tiles[h] = ot

        def _copy_out(dst, src, use_act):
            if use_act:
                nc.scalar.copy(dst, src)
            else:
                nc.vector.tensor_copy(dst, src)

        def _emit_wo_group(qc, qs, src_ot):
            # one q-subblock of the deferred output projection; interleaved
            # after each attention head so the PE fills ACT-limited stretches
            ysb_t = ysbp.tile([128, C], bf16, tag="ysb")
            row0 = qc * TCH + qs * 128
            for cc in range(C // 512):
                psy = pp.tile([128, 512], f32, tag="pp")
                for hh in range(HPC):
                    nc.tensor.matmul(
                        psy[:],
                        src_ot[hh][:, qs * 128:(qs + 1) * 128],
                        wo_tiles[hh][:, cc * 512:(cc + 1) * 512],
                        start=(hh == 0),
                        stop=(hh == HPC - 1),
                    )
                _copy_out(ysb_t[:, cc * 512:(cc + 1) * 512], psy[:], cc % 2 == 1)
                nc.sync.dma_start(
                    y[row0:row0 + 128, cc * 512:(cc + 1) * 512],
                    ysb_t[:, cc * 512:(cc + 1) * 512],
                )

        def _rope(ps, dst_h, tsl):
            # de-interleaved pairs: ps[0:64]=real, ps[64:128]=imag.
            # tA = ps*cos2 = [r*cos; i*cos]; tS = partition-swapped sin
            # products [i*sin; r*sin] (swap is free on the PSUM-reading muls;
            # SB+SB TensorTensor requires equal base partitions, PSUM input
            # is exempt). out_r = r*cos - i*sin, out_i = r*sin + i*cos.
            tA = rt.tile([128, TCH], f32, tag="rA")
            nc.vector.tensor_mul(tA[:], ps[:], cs2_t[:, tsl])
            tS = rt.tile([128, TCH], f32, tag="rB")
            nc.vector.tensor_mul(tS[0:64, :], ps[64:128, :], sn2_t[0:64, tsl])
            nc.vector.tensor_mul(tS[64:128, :], ps[0:64, :], sn2_t[64:128, tsl])
            nc.gpsimd.tensor_sub(dst_h[0:64, tsl], tA[0:64, :], tS[0:64, :])
            nc.gpsimd.tensor_add(dst_h[64:128, tsl], tS[64:128, :], tA[64:128, :])

        def _emit_qk_head_proj(t, h, xt):
            tsl_ = bass.ts(t, TCH)
            for wname, dst in (("wq", qT), ("wk", kT)):
                ps = pp.tile([128, TCH], f32, tag="pp")
                for ci in range(NCH):
                    nc.tensor.matmul(
                        ps[:],
                        w_tiles[(wname, ci)][:, h * 128:(h + 1) * 128],
                        xt[ci][:],
                        start=(ci == 0),
                        stop=(ci == NCH - 1),
                    )
                _rope(ps, dst[h], tsl_)

        def _emit_v_tsi(t, tsi, xt):
            ps = pp.tile([128, DH], f32, tag="pp")
            for ci in range(NCH):
                nc.tensor.matmul(
                    ps[:],
                    xt[ci][:, tsi * 128:(tsi + 1) * 128],
                    w_tiles[("wv", ci)][:],
                    start=(ci == 0),
                    stop=(ci == NCH - 1),
                )
            kb = t * 4 + tsi
            _copy_out(v_all[:, kb * DH:(kb + 1) * DH], ps[:], tsi % 2 == 1)

        def _emit_qk_proj_t0(xt):
            # run 7 projection groups concurrently across the (still idle)
            # attention psum pools so the PE keeps pace with the w/x DMA
            # stream: one matmul per group per arriving chunk. K-h3 is
            # emitted interleaved with the wv-paced V(0)-tsi0 group.
            tsl_ = bass.ts(0, TCH)
            _gp = [("wq", 0, pp, "pp"), ("wq", 1, pss, "s"),
                   ("wq", 2, pss, "s"), ("wq", 3, pss, "s"),
                   ("wk", 0, pso, "o"), ("wk", 1, pso, "o"),
                   ("wk", 2, pp, "pp")]
            groups = [
                (wn, h_, pool.tile([128, TCH], f32, tag=tg, name=f"g0_{wn}{h_}"))
                for wn, h_, pool, tg in _gp
            ]
            for ci in range(NCH):
                for wname, h, ps in groups:
                    nc.tensor.matmul(
                        ps[:],
                        w_tiles[(wname, ci)][:, h * 128:(h + 1) * 128],
                        xt[ci][:],
                        start=(ci == 0),
                        stop=(ci == NCH - 1),
                    )
            for wname, h, ps in groups:
                _rope(ps, (qT if wname == "wq" else kT)[h], tsl_)

        def _emit_v_proj_t0(xt):
            psk3 = pp.tile([128, TCH], f32, tag="pp", name="psk3")
            for tsi in range(TCH // 128):
                ps = pp.tile([128, DH], f32, tag="pp")
                for ci in range(NCH):
                    nc.tensor.matmul(
                        ps[:],
                        xt[ci][:, tsi * 128:(tsi + 1) * 128],
                        w_tiles[("wv", ci)][:],
                        start=(ci == 0),
                        stop=(ci == NCH - 1),
                    )
                    if tsi == 0:
                        nc.tensor.matmul(
                            psk3[:],
                            w_tiles[("wk", ci)][:, 3 * 128:4 * 128],
                            xt[ci][:],
                            start=(ci == 0), stop=(ci == NCH - 1),
                        )
                if tsi == 0:
                    _rope(psk3, kT[3], bass.ts(0, TCH))
                _copy_out(v_all[:, tsi * DH:(tsi + 1) * DH], ps[:], tsi % 2 == 1)

        def _prefetch_x(t):
            nsl = bass.ts(t, TCH)
            tiles = []
            for ci in range(NCH):
                x_ = xpool.tile([128, TCH], bf16, tag=f"x{ci}")
                eng = nc.scalar if ci % 2 == 0 else nc.sync
                eng.dma_start(x_[:], xT[ci * 128:(ci + 1) * 128, nsl])
                tiles.append(x_)
            return tiles

        # ---------------- t0: stream everything in ----------------
        # queue plan: sync=wq,consts,(x-odd prefetch),wo  scalar=x0,(x-even)
        #             gpsimd=wk,cos,sin  vector=wv
        xt_cur = []
        for ci in range(NCH):
            wt = wpool.tile([128, DH], bf16, tag=f"wq{ci}")
            nc.sync.dma_start(wt[:], wq[ci * 128:(ci + 1) * 128, :])
            w_tiles[("wq", ci)] = wt
            x_ = xpool.tile([128, TCH], bf16, tag=f"x{ci}")
            nc.scalar.dma_start(x_[:], xT[ci * 128:(ci + 1) * 128, 0:TCH])
            xt_cur.append(x_)
            wt = wpool.tile([128, DH], bf16, tag=f"wk{ci}")
            nc.gpsimd.dma_start(wt[:], wk[ci * 128:(ci + 1) * 128, :])
            w_tiles[("wk", ci)] = wt
            wt = wpool.tile([128, DH], bf16, tag=f"wv{ci}")
            eng = nc.sync if ci % 2 == 0 else nc.gpsimd
            eng.dma_start(wt[:], wv[ci * 128:(ci + 1) * 128, :])
            w_tiles[("wv", ci)] = wt
        nc.sync.dma_start(onesk[:], onesk_d)
        nc.sync.dma_start(ones1[:], ones1_d)
        nc.sync.dma_start(mb_t[:], mbd)
        nc.gpsimd.dma_start(cs2_t[:], cos2)
        nc.gpsimd.dma_start(sn2_t[:], sin2)
        _emit_qk_proj_t0(xt_cur)
        xt_next = _prefetch_x(1)
        _emit_v_proj_t0(xt_cur)
        for j in range(HPC):
            wt_ = wopool.tile([128, C], bf16, tag=f"wo{j}")
            nc.sync.dma_start(wt_[:], wo[j * 128:(j + 1) * 128, :])
            wo_tiles.append(wt_)

        # ------- steady pipeline: attn(t) with wo(t-1) + proj(t+1) ---------
        prev_ot = None
        for t in range(NT):
            if t >= 1:
                xt_cur = xt_next
                xt_next = _prefetch_x(t + 1) if t + 1 < NT else None
            qc = t
            kmax = 4 * qc + 3
            for h in range(HPC):
                q_sl = qT[h][:, qc * TCH:(qc + 1) * TCH]
                pso_t = pso.tile([128, TCH], f32, tag="o")
                psr_t = psr.tile([1, TCH], f32, tag="rs")
                av_q = []  # pending blocks for depth-2 pipelined AV/rowsum

                def _emit_av(h=h, pso_t=pso_t, psr_t=psr_t, kmax=kmax):
                    kb, qlo, e = av_q.pop(0)
                    nc.tensor.matmul(
                        pso_t[:, qlo:],
                        v_all[:, kb * DH + h * 128:kb * DH + (h + 1) * 128],
                        e[:, qlo:],
                        start=(kb == 0), stop=(kb == kmax),
                    )
                    nc.tensor.matmul(
                        psr_t[:, qlo:], onesk[:], e[:, qlo:],
                        start=(kb == 0), stop=(kb == kmax),
                    )

                for kb in range(kmax + 1):
                    i_rel = kb - 4 * qc
                    qlo = 128 * i_rel if i_rel > 0 else 0
                    pss_t = pss.tile([128, TCH], f32, tag="s")
                    nc.tensor.matmul(
                        pss_t[:, qlo:],
                        kT[h][:, kb * 128:(kb + 1) * 128],
                        q_sl[:, qlo:],
                        start=True,
                        stop=True,
                    )
                    e = ep.tile([128, TCH], bf16, tag="e")
                    nc.scalar.activation(
                        e[:, qlo:], pss_t[:, qlo:], AF.Exp, scale=SCALE
                    )
                    if i_rel >= 0:  # triangle mask on the diagonal square
                        nc.gpsimd.tensor_mul(
                            e[:, qlo:qlo + 128],
                            e[:, qlo:qlo + 128],
                            mb_t[:],
                        )
                    av_q.append((kb, qlo, e))
                    if kb == 1 and pending[0] is not None:
                        _emit_norm(*pending[0])
                        pending[0] = None
                    if len(av_q) > 3:
                        _emit_av()
                while av_q:
                    _emit_av()
                pending[0] = (h, pso_t, psr_t)
                if qc >= 1:
                    _emit_wo_group(qc - 1, h, prev_ot)
                if t + 1 < NT:
                    _emit_qk_head_proj(t + 1, h, xt_next)
                    _emit_v_tsi(t + 1, h, xt_next)
            _emit_norm(*pending[0])
            pending[0] = None
            prev_ot = dict(ot_tiles)
            ot_tiles = {}

        # ---------------- tail: wo for the last chunk ----------------
        for qs in range(TCH // 128):
            _emit_wo_group(NT - 1, qs, prev_ot)

    _split_waits(nc)
    return nc


_CACHED_NC = None


def _get_nc():
    global _CACHED_NC
    if _CACHED_NC is None:
        _CACHED_NC = _build_nc()
    return _CACHED_NC


# --------------------------------------------------------------------------
# host-side input prep / gather
# --------------------------------------------------------------------------
def _deinterleave_perm():
    """per-head column permutation: [2j for j<64] then [2j+1]"""
    p = np.empty(D, dtype=np.int64)
    p[:64] = np.arange(0, D, 2)
    p[64:] = np.arange(1, D, 2)
    return p


def _make_core_inputs(x, freqs_cos, freqs_sin, wq, wk, wv, wo):
    x = np.asarray(x, dtype=np.float32)
    freqs_cos = np.asarray(freqs_cos, dtype=np.float32)
    freqs_sin = np.asarray(freqs_sin, dtype=np.float32)
    wq = np.asarray(wq, dtype=np.float32)
    wk = np.asarray(wk, dtype=np.float32)
    wv = np.asarray(wv, dtype=np.float32)
    wo = np.asarray(wo, dtype=np.float32)

    perm = _deinterleave_perm()
    cosT = np.ascontiguousarray(freqs_cos.T)  # [64, T]
    sinT = np.ascontiguousarray(freqs_sin.T)
    cos2 = np.concatenate([cosT, cosT], axis=0).astype(BF16NP)  # [128, T]
    sin2 = np.concatenate([sinT, sinT], axis=0).astype(BF16NP)

    # causal triangle for the diagonal 128x128 square: mb[k, q] = 1 iff k <= q
    k_idx = np.arange(128)[:, None]
    q_idx = np.arange(128)[None, :]
    mb = (k_idx <= q_idx).astype(BF16NP)

    onesk = np.ones((128, 1), dtype=BF16NP)
    ones1 = np.ones((1, 128), dtype=np.float32)

    xTb = [np.ascontiguousarray(x[b].T).astype(BF16NP) for b in range(B)]

    in_maps = []
    for core in range(N_CORES):
        b, hg = core // 4, core % 4
        cols = slice(hg * DH, (hg + 1) * DH)
        wq_s = wq[:, cols].reshape(C, HPC, D)[:, :, perm].reshape(C, DH)
        wk_s = wk[:, cols].reshape(C, HPC, D)[:, :, perm].reshape(C, DH)
        in_maps.append({
            "xT": xTb[b],
            "wq": np.ascontiguousarray(wq_s).astype(BF16NP),
            "wk": np.ascontiguousarray(wk_s).astype(BF16NP),
            "wv": np.ascontiguousarray(wv[:, cols]).astype(BF16NP),
            "wo": np.ascontiguousarray(wo[cols, :]).astype(BF16NP),
            "cos2": cos2,
            "sin2": sin2,
            "mb": mb,
            "onesk": onesk,
            "ones1": ones1,
        })
    return in_maps


def kernel(x, freqs_cos, freqs_sin, wq, wk, wv, wo, _trace=False, _trace_kwargs=None):
    nc = _get_nc()
    in_maps = _make_core_inputs(x, freqs_cos, freqs_sin, wq, wk, wv, wo)
    res = run_bass_kernel_spmd(
        nc, in_maps, core_ids=list(range(N_CORES)), trace=_trace,
        **(_trace_kwargs or {}),
    )
    out = np.zeros((B, T, C), dtype=np.float32)
    for core in range(N_CORES):
        out[core // 4] += np.asarray(res.results[core]["y"], dtype=np.float32)
    if _trace:
        kernel.last_results = res
    return out
